# revision 23
# baseline (speedup 1.0000x reference)
"""Trainium2 Bass kernel for ConvFourierKANLayer.

Computes y = conv2d(cos(x*k), w0) + conv2d(sin(x*k), w1) + bias for
k = 1..10 (G=10 Fourier orders), 3x3 kernel, pad 1, C=64 -> O=128.

Strategy (8 NeuronCores, data-parallel over batch B=16 -> 2 per core):
  - F(2,3) Winograd along H: the kh tap dimension collapses into 4
    transformed planes per 2 output rows (2 m-values/output instead of
    3 taps), cutting streamed PE rows 1.5x. The dw taps stay spatial
    (free AP shifts). Weight transform G=[[1,0,0],[.5,.5,.5],
    [.5,-.5,.5],[0,0,1]] is folded into the host-side weight repack.
  - Matmuls run in bf16 (1 cyc/row, and enables DVE 2x_1P mode for the
    bf16 transform subtracts/adds, which fp32 TT does not get).
  - Trig gen per g-pair j (partitions = 2 g's x 64 c):
      u  = x*(k/2pi) + 16          (ACT Identity, per-partition scale)
      v  = (u + 2^23) - 2^23       (DVE fused tensor_scalar round)
      w  = u - v  in [-0.5, 0.5]   (DVE tensor_tensor)
      a  = |w|                     (ACT Abs)
      sin(kx) = Sin(2pi * w)       (ACT spline, arg in [-pi, pi])
      cos(kx) = Sin(pi/2 - 2pi*a)  (ACT spline, arg in [-pi/2, pi/2])
  - Per 16-row output chunk: 4 PSUM banks accumulate m0..m3 planes
    over (j, branch, dw) = 30 matmuls each of N=512 rows; inverse
    transform y_even = m0+m1+m2+bias, y_odd = m1-m2-m3+bias runs on
    ACT (PSUM->SBUF copies) + DVE scalar_tensor_tensor.
"""

import numpy as np

import concourse.bass as bass
import concourse.mybir as mybir
import concourse.tile as tile
from concourse import bacc
from concourse.bass_utils import run_bass_kernel_spmd

N_CORES = 8
B, C, H, W = 16, 64, 64, 64
O = 128
G = 10
BS = B // N_CORES  # batches per core
HT = 16            # output rows per chunk (4 psum banks of 8 row-pairs)
NT = 4 * 5 * 2 * 3  # weight tiles: i x j x branch x dw = 120

PI = float(np.pi)
TWO_PI = float(2 * np.pi)
HALF_PI = float(np.pi / 2)
MAGIC = 8388608.0  # 2^23: fp32 round-to-nearest-integer magic constant

F32 = mybir.dt.float32
BF16 = mybir.dt.bfloat16

_CACHE = {}


def _build_module(reps=1, ht=HT):
    nc = bacc.Bacc("TRN2", target_bir_lowering=False)
    x_d = nc.dram_tensor("x", [BS, C, H, W], F32, kind="ExternalInput")
    w_d = nc.dram_tensor("w", [128, NT, 128], BF16, kind="ExternalInput")
    kv_d = nc.dram_tensor("kvec", [128, 5], F32, kind="ExternalInput")
    bias_d = nc.dram_tensor("biasv", [128, 1], F32, kind="ExternalInput")
    y_d = nc.dram_tensor("y", [BS, O, H, W], F32, kind="ExternalOutput")

    add = mybir.AluOpType.add
    sub = mybir.AluOpType.subtract
    mult = mybir.AluOpType.mult
    sin_f = mybir.ActivationFunctionType.Sin
    abs_f = mybir.ActivationFunctionType.Abs
    id_f = mybir.ActivationFunctionType.Identity

    IR = ht + 2  # input rows per chunk (halo of 1 above/below)
    RP = ht // 2  # row pairs

    with tile.TileContext(nc) as tc:
        CS_BUFS = 3
        V_BUFS = 3
        with (
            tc.tile_pool(name="const", bufs=1) as cpool,
            tc.tile_pool(name="wpool", bufs=1) as wpool,
            tc.tile_pool(name="gen", bufs=3) as gen,
            tc.tile_pool(name="cspool", bufs=CS_BUFS) as cspool,
            tc.tile_pool(name="vpool", bufs=V_BUFS) as vpool,
            tc.tile_pool(name="inv", bufs=3) as inv,
            tc.tile_pool(name="outp", bufs=3) as outp,
            tc.tile_pool(name="psum", bufs=2, space="PSUM") as psum,
        ):
            wt = wpool.tile([128, NT, 128], BF16)
            for wi in range(0, NT, 20):
                nc.sync.dma_start(wt[:, wi : wi + 20, :], w_d[:, wi : wi + 20, :])
            kvt = cpool.tile([128, 5], F32)
            nc.sync.dma_start(kvt[:], kv_d[:])
            bt = cpool.tile([128, 1], F32)
            nc.sync.dma_start(bt[:], bias_d[:])
            b16 = cpool.tile([128, 1], F32)
            nc.vector.memset(b16[:], 16.0)
            bhpi = cpool.tile([128, 1], F32)
            nc.vector.memset(bhpi[:], HALF_PI)

            def emit_inverse(pend):
                # y_even = m0+m1+m2+b, y_odd = m1-m2-m3+b
                pb, pr0, ps = pend
                t2 = inv.tile([128, RP, 64], F32, tag="t2")
                nc.scalar.activation(t2[:], ps[2][:], id_f)
                t12 = inv.tile([128, RP, 64], F32, tag="t12")
                nc.vector.scalar_tensor_tensor(
                    t12[:], ps[1][:], bt[:, 0:1], t2[:], add, add
                )
                t12m = inv.tile([128, RP, 64], F32, tag="t12m")
                nc.vector.scalar_tensor_tensor(
                    t12m[:], ps[1][:], bt[:, 0:1], t2[:], add, sub
                )
                yb = outp.tile([128, ht, 64], F32, tag="yb")
                nc.vector.scalar_tensor_tensor(
                    yb[:, 0:ht:2, :], ps[0][:], 0.0, t12[:], add, add
                )
                nc.vector.scalar_tensor_tensor(
                    yb[:, 1:ht:2, :], ps[3][:], -1.0, t12m[:], mult, add
                )
                nc.sync.dma_start(y_d[pb, :, pr0 : pr0 + ht, :], yb[:])

            pending = None
            it = 0  # global (chunk, j) iteration counter for border init
            for rep in range(reps):
              for b in range(BS):
                for r0 in range(0, H, ht):
                    gr0, gr1 = max(0, r0 - 1), min(H, r0 + ht + 1)
                    l0 = gr0 - (r0 - 1)  # local row of first real row (0/1)
                    nrows = gr1 - gr0
                    rs = slice(l0, l0 + nrows)

                    xd = gen.tile([128, IR, W], F32, tag="xd")
                    if l0 == 1:
                        nc.gpsimd.memset(xd[:, 0:1, :], 0)
                    if gr1 == H and l0 + nrows < IR:
                        nc.gpsimd.memset(xd[:, IR - 1 : IR, :], 0)
                    nc.sync.dma_start(xd[0:64, rs, :], x_d[b, :, gr0:gr1, :])
                    nc.sync.dma_start(xd[64:128, rs, :], x_d[b, :, gr0:gr1, :])

                    pss = [
                        psum.tile([128, RP, 64], F32, tag=f"ps{i}",
                                  name=f"ps{i}_{rep}_{b}_{r0}")
                        for i in range(4)
                    ]

                    for j in range(5):
                        u = gen.tile([128, IR, W], F32, tag="u")
                        nc.gpsimd.tensor_scalar(
                            u[:], xd[:], kvt[:, j : j + 1], 16.0, mult, add
                        )
                        v = gen.tile([128, IR, W], F32, tag="v")
                        nc.vector.tensor_scalar(v[:], u[:], MAGIC, MAGIC, add, sub)
                        w = gen.tile([128, IR, W], F32, tag="w")
                        nc.vector.tensor_sub(w[:], u[:], v[:])
                        a = gen.tile([128, IR, W], F32, tag="a")
                        nc.scalar.activation(a[:], w[:], abs_f)

                        st = cspool.tile([128, IR, W + 4], BF16, tag="st")
                        ct = cspool.tile([128, IR, W + 4], BF16, tag="ct")
                        for z in (st, ct):
                            nc.gpsimd.memset(z[:, :, 0:2], 0)
                            nc.gpsimd.memset(z[:, :, W + 2 : W + 4], 0)
                        nc.scalar.activation(
                            st[:, :, 2 : W + 2], w[:], sin_f, scale=TWO_PI
                        )
                        nc.scalar.activation(
                            ct[:, :, 2 : W + 2], a[:], sin_f,
                            bias=bhpi[:], scale=-TWO_PI,
                        )
                        # cos of zero-padded halo rows must be 0, not 1
                        if l0 == 1:
                            nc.gpsimd.memset(ct[:, 0:1, :], 0)
                        if gr1 == H and l0 + nrows < IR:
                            nc.gpsimd.memset(ct[:, IR - 1 : IR, :], 0)

                        vc = vpool.tile([128, 4, RP, W + 4], BF16, tag="vc")
                        vs = vpool.tile([128, 4, RP, W + 4], BF16, tag="vs")
                        for src, vt in ((ct, vc), (st, vs)):
                            nc.gpsimd.memset(vt[:, :, :, 0:2], 0)
                            nc.gpsimd.memset(vt[:, :, :, W + 2 : W + 4], 0)
                            cs = slice(2, W + 2)
                            d0 = src[:, 0 : IR - 2 : 2, cs]
                            d1 = src[:, 1 : IR - 1 : 2, cs]
                            d2 = src[:, 2:IR:2, cs]
                            d3 = src[:, 3:IR:2, cs]
                            nc.vector.tensor_sub(vt[:, 0, :, cs], d0, d2)
                            nc.vector.tensor_add(vt[:, 1, :, cs], d1, d2)
                            nc.vector.tensor_sub(vt[:, 2, :, cs], d2, d1)
                            v3eng = nc.gpsimd if vt is vs else nc.vector
                            v3eng.tensor_sub(vt[:, 3, :, cs], d1, d3)
                        it += 1

                        for i in range(4):
                            for br, vt in ((0, vc), (1, vs)):
                                for dw in range(3):
                                    t_idx = ((j * 4 + i) * 2 + br) * 3 + dw
                                    nc.tensor.matmul(
                                        pss[i][:],
                                        wt[:, t_idx, :],
                                        vt[:, i, :, dw + 1 : dw + 65],
                                        start=(j == 0 and br == 0 and dw == 0),
                                        stop=(j == 4 and br == 1 and dw == 2),
                                    )

                        # previous chunk's inverse, deferred so it doesn't
                        # head-of-line-block this chunk's DVE/ACT gen work
                        if j == 2 and pending is not None:
                            emit_inverse(pending)
                            pending = None

                    pending = (b, r0, pss)
            if pending is not None:
                emit_inverse(pending)
    nc.finalize()
    return nc


def _get_module(reps=1, ht=HT):
    key = ("nc", reps, ht)
    if key not in _CACHE:
        _CACHE[key] = _build_module(reps, ht)
    return _CACHE[key]


def _host_weights(fc, mmdt=None):
    # fc: (2, O, C, kH, kW, G) -> winograd-H transform along kH, then pack
    # lhsT[p=(gp*64+c), t=((i*5+j)*2+br)*3+dw, o]
    import ml_dtypes

    Gw = np.array(
        [[1, 0, 0], [0.5, 0.5, 0.5], [0.5, -0.5, 0.5], [0, 0, 1]], np.float32
    )
    wt6 = np.einsum("iK,aocKVg->aociVg", Gw, fc)  # (2, O, C, 4, 3, G)
    wt7 = wt6.reshape(2, O, C, 4, 3, 5, 2)  # g -> (j, gp)
    # -> (gp, c, j, i, br, dw, o): j-major so the first chunk's weight
    # tiles land early in the startup DMA
    wt8 = np.transpose(wt7, (6, 2, 5, 3, 0, 4, 1))
    return np.ascontiguousarray(
        wt8.reshape(128, NT, 128).astype(ml_dtypes.bfloat16)
    )


def _host_kvec():
    kvec = np.zeros((128, 5), np.float32)
    for j in range(5):
        kvec[0:64, j] = (2 * j + 1) / TWO_PI
        kvec[64:128, j] = (2 * j + 2) / TWO_PI
    return kvec


def kernel(x, fouriercoeffs, bias):
    x = np.ascontiguousarray(np.asarray(x, dtype=np.float32))
    fc = np.asarray(fouriercoeffs, dtype=np.float32)
    w_host = _host_weights(fc)
    kvec = _host_kvec()
    biasv = np.ascontiguousarray(
        np.asarray(bias, dtype=np.float32).reshape(128, 1)
    )

    nc = _get_module()
    in_maps = [
        {"x": x[i * BS : (i + 1) * BS], "w": w_host, "kvec": kvec, "biasv": biasv}
        for i in range(N_CORES)
    ]
    res = run_bass_kernel_spmd(nc, in_maps, list(range(N_CORES))).results
    return np.concatenate([res[i]["y"] for i in range(N_CORES)], axis=0)


# revision 28
# speedup vs baseline: 1.1200x; 1.1200x over previous
"""Trainium2 Bass kernel for ConvFourierKANLayer.

Computes y = conv2d(cos(x*k), w0) + conv2d(sin(x*k), w1) + bias for
k = 1..10 (G=10 Fourier orders), 3x3 kernel, pad 1, C=64 -> O=128.

Strategy (8 NeuronCores, data-parallel over batch B=16 -> 2 per core):
  - F(2,3) Winograd along H: the kh tap dimension collapses into 4
    transformed planes per 2 output rows (2 m-values/output instead of
    3 taps), cutting streamed PE rows 1.5x. The dw taps stay spatial
    (free AP shifts). Weight transform G=[[1,0,0],[.5,.5,.5],
    [.5,-.5,.5],[0,0,1]] is folded into the host-side weight repack.
  - Matmuls run in bf16 (1 cyc/row, and enables DVE 2x_1P mode for the
    bf16 transform subtracts/adds, which fp32 TT does not get).
  - Trig gen per g-pair j (partitions = 2 g's x 64 c):
      u  = x*(k/2pi) + 16          (ACT Identity, per-partition scale)
      v  = (u + 2^23) - 2^23       (DVE fused tensor_scalar round)
      w  = u - v  in [-0.5, 0.5]   (DVE tensor_tensor)
      a  = |w|                     (ACT Abs)
      sin(kx) = Sin(2pi * w)       (ACT spline, arg in [-pi, pi])
      cos(kx) = Sin(pi/2 - 2pi*a)  (ACT spline, arg in [-pi/2, pi/2])
  - Per 16-row output chunk: 4 PSUM banks accumulate m0..m3 planes
    over (j, branch, dw) = 30 matmuls each of N=512 rows; inverse
    transform y_even = m0+m1+m2+bias, y_odd = m1-m2-m3+bias runs on
    ACT (PSUM->SBUF copies) + DVE scalar_tensor_tensor.
"""

import numpy as np

import concourse.bass as bass
import concourse.mybir as mybir
import concourse.tile as tile
from concourse import bacc
from concourse.bass_utils import run_bass_kernel_spmd

N_CORES = 8
B, C, H, W = 16, 64, 64, 64
O = 128
G = 10
BS = B // N_CORES  # batches per core
HT = 16            # output rows per chunk (4 psum banks of 8 row-pairs)
NT = 4 * 5 * 2 * 3  # weight tiles: i x j x branch x dw = 120

PI = float(np.pi)
TWO_PI = float(2 * np.pi)
HALF_PI = float(np.pi / 2)
MAGIC = 8388608.0  # 2^23: fp32 round-to-nearest-integer magic constant

F32 = mybir.dt.float32
BF16 = mybir.dt.bfloat16

_CACHE = {}


def _build_module(reps=1, ht=HT):
    nc = bacc.Bacc("TRN2", target_bir_lowering=False)
    x_d = nc.dram_tensor("x", [BS, C, H, W], F32, kind="ExternalInput")
    w_d = nc.dram_tensor("w", [128, NT, 128], BF16, kind="ExternalInput")
    kv_d = nc.dram_tensor("kvec", [128, 5], F32, kind="ExternalInput")
    bias_d = nc.dram_tensor("biasv", [128, 1], F32, kind="ExternalInput")
    y_d = nc.dram_tensor("y", [BS, O, H, W], F32, kind="ExternalOutput")

    add = mybir.AluOpType.add
    sub = mybir.AluOpType.subtract
    mult = mybir.AluOpType.mult
    sin_f = mybir.ActivationFunctionType.Sin
    abs_f = mybir.ActivationFunctionType.Abs
    id_f = mybir.ActivationFunctionType.Identity

    IR = ht + 2  # input rows per chunk (halo of 1 above/below)
    RP = ht // 2  # row pairs

    with tile.TileContext(nc) as tc:
        CS_BUFS = 3
        V_BUFS = 3
        with (
            tc.tile_pool(name="const", bufs=1) as cpool,
            tc.tile_pool(name="wpool", bufs=1) as wpool,
            tc.tile_pool(name="gen", bufs=3) as gen,
            tc.tile_pool(name="cspool", bufs=CS_BUFS) as cspool,
            tc.tile_pool(name="vpool", bufs=V_BUFS) as vpool,
            tc.tile_pool(name="inv", bufs=3) as inv,
            tc.tile_pool(name="outp", bufs=3) as outp,
            tc.tile_pool(name="psum", bufs=2, space="PSUM") as psum,
        ):
            wt = wpool.tile([128, NT, 128], BF16)
            for wi in range(0, NT, 20):
                nc.sync.dma_start(wt[:, wi : wi + 20, :], w_d[:, wi : wi + 20, :])
            kvt = cpool.tile([128, 5], F32)
            nc.sync.dma_start(kvt[:], kv_d[:])
            bt = cpool.tile([128, 1], F32)
            nc.sync.dma_start(bt[:], bias_d[:])
            b16 = cpool.tile([128, 1], F32)
            nc.vector.memset(b16[:], 16.0)
            bhpi = cpool.tile([128, 1], F32)
            nc.vector.memset(bhpi[:], HALF_PI)

            def emit_inverse(pend):
                # y_even = m0+m1+m2+b, y_odd = m1-m2-m3+b
                pb, pr0, ps = pend
                t2 = inv.tile([128, RP, 64], F32, tag="t2")
                nc.scalar.activation(t2[:], ps[2][:], id_f)
                t12 = inv.tile([128, RP, 64], F32, tag="t12")
                nc.vector.scalar_tensor_tensor(
                    t12[:], ps[1][:], bt[:, 0:1], t2[:], add, add
                )
                t12m = inv.tile([128, RP, 64], F32, tag="t12m")
                nc.vector.scalar_tensor_tensor(
                    t12m[:], ps[1][:], bt[:, 0:1], t2[:], add, sub
                )
                yb = outp.tile([128, ht, 64], F32, tag="yb")
                nc.vector.scalar_tensor_tensor(
                    yb[:, 0:ht:2, :], ps[0][:], 0.0, t12[:], add, add
                )
                nc.vector.scalar_tensor_tensor(
                    yb[:, 1:ht:2, :], ps[3][:], -1.0, t12m[:], mult, add
                )
                nc.sync.dma_start(y_d[pb, :, pr0 : pr0 + ht, :], yb[:])

            pending = None
            it = 0  # global (chunk, j) iteration counter for border init
            for rep in range(reps):
              for b in range(BS):
                for r0 in range(0, H, ht):
                    gr0, gr1 = max(0, r0 - 1), min(H, r0 + ht + 1)
                    l0 = gr0 - (r0 - 1)  # local row of first real row (0/1)
                    nrows = gr1 - gr0
                    rs = slice(l0, l0 + nrows)

                    xd = gen.tile([128, IR, W], F32, tag="xd")
                    if l0 == 1:
                        nc.gpsimd.memset(xd[:, 0:1, :], 0)
                    if gr1 == H and l0 + nrows < IR:
                        nc.gpsimd.memset(xd[:, IR - 1 : IR, :], 0)
                    nc.sync.dma_start(xd[0:64, rs, :], x_d[b, :, gr0:gr1, :])
                    nc.sync.dma_start(xd[64:128, rs, :], x_d[b, :, gr0:gr1, :])

                    pss = [
                        psum.tile([128, RP, 64], F32, tag=f"ps{i}",
                                  name=f"ps{i}_{rep}_{b}_{r0}")
                        for i in range(4)
                    ]

                    def emit_u(j):
                        ut = gen.tile([128, IR, W], F32, tag="u")
                        nc.gpsimd.tensor_scalar(
                            ut[:], xd[:], kvt[:, j : j + 1], 16.0, mult, add
                        )
                        return ut

                    # u(j+1) is emitted during iteration j so it sits ahead
                    # of v3(j) in the Pool FIFO (avoids head-of-line wait)
                    u_next = emit_u(0)
                    for j in range(5):
                        u = u_next
                        if j < 4:
                            u_next = emit_u(j + 1)
                        v = gen.tile([128, IR, W], F32, tag="v")
                        nc.vector.tensor_scalar(v[:], u[:], MAGIC, MAGIC, add, sub)
                        w = gen.tile([128, IR, W], F32, tag="w")
                        nc.vector.tensor_sub(w[:], u[:], v[:])
                        a = gen.tile([128, IR, W], F32, tag="a")
                        nc.scalar.activation(a[:], w[:], abs_f)

                        st = cspool.tile([128, IR, W], BF16, tag="st")
                        ct = cspool.tile([128, IR, W], BF16, tag="ct")
                        nc.scalar.activation(st[:], w[:], sin_f, scale=TWO_PI)
                        nc.scalar.activation(
                            ct[:], a[:], sin_f, bias=bhpi[:], scale=-TWO_PI
                        )
                        # cos of zero-padded halo rows must be 0, not 1
                        if l0 == 1:
                            nc.gpsimd.memset(ct[:, 0:1, :], 0)
                        if gr1 == H and l0 + nrows < IR:
                            nc.gpsimd.memset(ct[:, IR - 1 : IR, :], 0)

                        vc = vpool.tile([128, 4, RP, W + 4], BF16, tag="vc")
                        vs = vpool.tile([128, 4, RP, W + 4], BF16, tag="vs")
                        for src, vt in ((ct, vc), (st, vs)):
                            nc.gpsimd.memset(vt[:, :, :, 0:2], 0)
                            nc.gpsimd.memset(vt[:, :, :, W + 2 : W + 4], 0)
                            cs = slice(2, W + 2)
                            d0 = src[:, 0 : IR - 2 : 2, :]
                            d1 = src[:, 1 : IR - 1 : 2, :]
                            d2 = src[:, 2:IR:2, :]
                            d3 = src[:, 3:IR:2, :]
                            nc.vector.tensor_sub(vt[:, 0, :, cs], d0, d2)
                            nc.vector.tensor_add(vt[:, 1, :, cs], d1, d2)
                            nc.vector.tensor_sub(vt[:, 2, :, cs], d2, d1)
                            v3eng = nc.gpsimd if vt is vs else nc.vector
                            v3eng.tensor_sub(vt[:, 3, :, cs], d1, d3)
                        it += 1

                        for i in range(4):
                            for br, vt in ((0, vc), (1, vs)):
                                for dw in range(3):
                                    t_idx = ((j * 4 + i) * 2 + br) * 3 + dw
                                    nc.tensor.matmul(
                                        pss[i][:],
                                        wt[:, t_idx, :],
                                        vt[:, i, :, dw + 1 : dw + 65],
                                        start=(j == 0 and br == 0 and dw == 0),
                                        stop=(j == 4 and br == 1 and dw == 2),
                                    )

                        # previous chunk's inverse, deferred so it doesn't
                        # head-of-line-block this chunk's DVE/ACT gen work
                        if j == 2 and pending is not None:
                            emit_inverse(pending)
                            pending = None

                    pending = (b, r0, pss)
            if pending is not None:
                emit_inverse(pending)
    nc.finalize()
    return nc


def _get_module(reps=1, ht=HT):
    key = ("nc", reps, ht)
    if key not in _CACHE:
        _CACHE[key] = _build_module(reps, ht)
    return _CACHE[key]


def _host_weights(fc, mmdt=None):
    # fc: (2, O, C, kH, kW, G) -> winograd-H transform along kH, then pack
    # lhsT[p=(gp*64+c), t=((i*5+j)*2+br)*3+dw, o]
    import ml_dtypes

    Gw = np.array(
        [[1, 0, 0], [0.5, 0.5, 0.5], [0.5, -0.5, 0.5], [0, 0, 1]], np.float32
    )
    wt6 = np.einsum("iK,aocKVg->aociVg", Gw, fc)  # (2, O, C, 4, 3, G)
    wt7 = wt6.reshape(2, O, C, 4, 3, 5, 2)  # g -> (j, gp)
    # -> (gp, c, j, i, br, dw, o): j-major so the first chunk's weight
    # tiles land early in the startup DMA
    wt8 = np.transpose(wt7, (6, 2, 5, 3, 0, 4, 1))
    return np.ascontiguousarray(
        wt8.reshape(128, NT, 128).astype(ml_dtypes.bfloat16)
    )


def _host_kvec():
    kvec = np.zeros((128, 5), np.float32)
    for j in range(5):
        kvec[0:64, j] = (2 * j + 1) / TWO_PI
        kvec[64:128, j] = (2 * j + 2) / TWO_PI
    return kvec


def kernel(x, fouriercoeffs, bias):
    x = np.ascontiguousarray(np.asarray(x, dtype=np.float32))
    fc = np.asarray(fouriercoeffs, dtype=np.float32)
    w_host = _host_weights(fc)
    kvec = _host_kvec()
    biasv = np.ascontiguousarray(
        np.asarray(bias, dtype=np.float32).reshape(128, 1)
    )

    nc = _get_module()
    in_maps = [
        {"x": x[i * BS : (i + 1) * BS], "w": w_host, "kvec": kvec, "biasv": biasv}
        for i in range(N_CORES)
    ]
    res = run_bass_kernel_spmd(nc, in_maps, list(range(N_CORES))).results
    return np.concatenate([res[i]["y"] for i in range(N_CORES)], axis=0)


# revision 29
# speedup vs baseline: 1.3108x; 1.1703x over previous
"""Trainium2 Bass kernel for ConvFourierKANLayer.

Computes y = conv2d(cos(x*k), w0) + conv2d(sin(x*k), w1) + bias for
k = 1..10 (G=10 Fourier orders), 3x3 kernel, pad 1, C=64 -> O=128.

Strategy (8 NeuronCores, data-parallel over batch B=16 -> 2 per core):
  - F(2,3) Winograd along H: the kh tap dimension collapses into 4
    transformed planes per 2 output rows (2 m-values/output instead of
    3 taps), cutting streamed PE rows 1.5x. The dw taps stay spatial
    (free AP shifts). Weight transform G=[[1,0,0],[.5,.5,.5],
    [.5,-.5,.5],[0,0,1]] is folded into the host-side weight repack.
  - Matmuls run in bf16 (1 cyc/row, and enables DVE 2x_1P mode for the
    bf16 transform subtracts/adds, which fp32 TT does not get).
  - Trig gen per g-pair j (partitions = 2 g's x 64 c), spread across
    engines so the PE stays the sole bottleneck:
      u  = x*(k/2pi) + 16          (GPSIMD tensor_scalar, per-part k)
      v  = (u + 2^23) - 2^23       (DVE fused tensor_scalar round)
      w  = u - v  in [-0.5, 0.5]   (DVE tensor_tensor)
      a  = |w|                     (ACT Abs)
      sin(kx) = Sin(2pi * w)       (ACT spline, arg in [-pi, pi])
      cos(kx) = Sin(pi/2 - 2pi*a)  (ACT spline, arg in [-pi/2, pi/2])
  - Per 16-row output chunk: 4 PSUM banks accumulate m0..m3 planes
    over (j, branch, dw) = 30 matmuls each of N=512 rows; inverse
    transform y_even = m0+m1+m2+bias, y_odd = m1-m2-m3+bias runs on
    ACT (PSUM->SBUF copies) + DVE scalar_tensor_tensor.
"""

import numpy as np

import concourse.bass as bass
import concourse.mybir as mybir
import concourse.tile as tile
from concourse import bacc
from concourse.bass_utils import run_bass_kernel_spmd

N_CORES = 8
B, C, H, W = 16, 64, 64, 64
O = 128
G = 10
BS = B // N_CORES  # batches per core
HT = 16            # output rows per chunk (4 psum banks of 8 row-pairs)
NT = 4 * 5 * 2 * 3  # weight tiles: i x j x branch x dw = 120

PI = float(np.pi)
TWO_PI = float(2 * np.pi)
HALF_PI = float(np.pi / 2)
MAGIC = 8388608.0  # 2^23: fp32 round-to-nearest-integer magic constant

F32 = mybir.dt.float32
BF16 = mybir.dt.bfloat16

_CACHE = {}


def _build_module(reps=1, ht=HT):
    nc = bacc.Bacc("TRN2", target_bir_lowering=False)
    x_d = nc.dram_tensor("x", [BS, C, H, W], F32, kind="ExternalInput")
    w_d = nc.dram_tensor("w", [128, NT, 128], BF16, kind="ExternalInput")
    kv_d = nc.dram_tensor("kvec", [128, 5], F32, kind="ExternalInput")
    bias_d = nc.dram_tensor("biasv", [128, 1], F32, kind="ExternalInput")
    y_d = nc.dram_tensor("y", [BS, O, H, W], F32, kind="ExternalOutput")

    add = mybir.AluOpType.add
    sub = mybir.AluOpType.subtract
    mult = mybir.AluOpType.mult
    sin_f = mybir.ActivationFunctionType.Sin
    abs_f = mybir.ActivationFunctionType.Abs
    id_f = mybir.ActivationFunctionType.Identity

    IR = ht + 2  # input rows per chunk (halo of 1 above/below)
    RP = ht // 2  # row pairs

    with tile.TileContext(nc) as tc:
        CS_BUFS = 3
        V_BUFS = 3
        with (
            tc.tile_pool(name="const", bufs=1) as cpool,
            tc.tile_pool(name="wpool", bufs=1) as wpool,
            tc.tile_pool(name="gen", bufs=3) as gen,
            tc.tile_pool(name="cspool", bufs=CS_BUFS) as cspool,
            tc.tile_pool(name="vpool", bufs=V_BUFS) as vpool,
            tc.tile_pool(name="inv", bufs=3) as inv,
            tc.tile_pool(name="outp", bufs=3) as outp,
            tc.tile_pool(name="psum", bufs=2, space="PSUM") as psum,
        ):
            wt = wpool.tile([128, NT, 128], BF16)
            for wi in range(0, NT, 20):
                nc.sync.dma_start(wt[:, wi : wi + 20, :], w_d[:, wi : wi + 20, :])
            kvt = cpool.tile([128, 5], F32)
            nc.sync.dma_start(kvt[:], kv_d[:])
            bt = cpool.tile([128, 1], F32)
            nc.sync.dma_start(bt[:], bias_d[:])
            b16 = cpool.tile([128, 1], F32)
            nc.vector.memset(b16[:], 16.0)
            bhpi = cpool.tile([128, 1], F32)
            nc.vector.memset(bhpi[:], HALF_PI)

            def emit_inverse(pend):
                # y_even = m0+m1+m2+b, y_odd = m1-m2-m3+b
                pb, pr0, ps = pend
                t2 = inv.tile([128, RP, 64], F32, tag="t2")
                nc.scalar.activation(t2[:], ps[2][:], id_f)
                t12 = inv.tile([128, RP, 64], F32, tag="t12")
                nc.vector.scalar_tensor_tensor(
                    t12[:], ps[1][:], bt[:, 0:1], t2[:], add, add
                )
                t12m = inv.tile([128, RP, 64], F32, tag="t12m")
                nc.vector.scalar_tensor_tensor(
                    t12m[:], ps[1][:], bt[:, 0:1], t2[:], add, sub
                )
                yb = outp.tile([128, ht, 64], F32, tag="yb")
                nc.vector.scalar_tensor_tensor(
                    yb[:, 0:ht:2, :], ps[0][:], 0.0, t12[:], add, add
                )
                nc.vector.scalar_tensor_tensor(
                    yb[:, 1:ht:2, :], ps[3][:], -1.0, t12m[:], mult, add
                )
                nc.sync.dma_start(y_d[pb, :, pr0 : pr0 + ht, :], yb[:])

            pending = None
            it = 0  # global (chunk, j) iteration counter for border init
            for rep in range(reps):
              for b in range(BS):
                for r0 in range(0, H, ht):
                    gr0, gr1 = max(0, r0 - 1), min(H, r0 + ht + 1)
                    l0 = gr0 - (r0 - 1)  # local row of first real row (0/1)
                    nrows = gr1 - gr0
                    rs = slice(l0, l0 + nrows)

                    xd = gen.tile([128, IR, W], F32, tag="xd")
                    if l0 == 1:
                        nc.gpsimd.memset(xd[:, 0:1, :], 0)
                    if gr1 == H and l0 + nrows < IR:
                        nc.gpsimd.memset(xd[:, IR - 1 : IR, :], 0)
                    nc.sync.dma_start(xd[0:64, rs, :], x_d[b, :, gr0:gr1, :])
                    nc.sync.dma_start(xd[64:128, rs, :], x_d[b, :, gr0:gr1, :])

                    pss = [
                        psum.tile([128, RP, 64], F32, tag=f"ps{i}",
                                  name=f"ps{i}_{rep}_{b}_{r0}")
                        for i in range(4)
                    ]

                    def emit_u(j):
                        ut = gen.tile([128, IR, W], F32, tag="u")
                        nc.gpsimd.tensor_scalar(
                            ut[:], xd[:], kvt[:, j : j + 1], 16.0, mult, add
                        )
                        return ut

                    # u(j+1) is emitted during iteration j so it sits ahead
                    # of v3(j) in the Pool FIFO (avoids head-of-line wait)
                    u_next = emit_u(0)
                    for j in range(5):
                        u = u_next
                        if j < 4:
                            u_next = emit_u(j + 1)
                        v = gen.tile([128, IR, W], F32, tag="v")
                        nc.vector.tensor_scalar(v[:], u[:], MAGIC, MAGIC, add, sub)
                        w = gen.tile([128, IR, W], F32, tag="w")
                        nc.vector.tensor_sub(w[:], u[:], v[:])
                        a = gen.tile([128, IR, W], F32, tag="a")
                        nc.scalar.activation(a[:], w[:], abs_f)

                        st = cspool.tile([128, IR, W], BF16, tag="st")
                        ct = cspool.tile([128, IR, W], BF16, tag="ct")
                        nc.scalar.activation(st[:], w[:], sin_f, scale=TWO_PI)
                        nc.scalar.activation(
                            ct[:], a[:], sin_f, bias=bhpi[:], scale=-TWO_PI
                        )
                        # cos of zero-padded halo rows must be 0, not 1
                        if l0 == 1:
                            nc.gpsimd.memset(ct[:, 0:1, :], 0)
                        if gr1 == H and l0 + nrows < IR:
                            nc.gpsimd.memset(ct[:, IR - 1 : IR, :], 0)

                        vc = vpool.tile([128, 4, RP, W + 4], BF16, tag="vc")
                        vs = vpool.tile([128, 4, RP, W + 4], BF16, tag="vs")
                        for src, vt in ((ct, vc), (st, vs)):
                            nc.gpsimd.memset(vt[:, :, :, 0:2], 0)
                            nc.gpsimd.memset(vt[:, :, :, W + 2 : W + 4], 0)
                            cs = slice(2, W + 2)
                            d0 = src[:, 0 : IR - 2 : 2, :]
                            d1 = src[:, 1 : IR - 1 : 2, :]
                            d2 = src[:, 2:IR:2, :]
                            d3 = src[:, 3:IR:2, :]
                            nc.vector.tensor_sub(vt[:, 0, :, cs], d0, d2)
                            nc.vector.tensor_add(vt[:, 1, :, cs], d1, d2)
                            nc.vector.tensor_sub(vt[:, 2, :, cs], d2, d1)
                            v3eng = nc.gpsimd if vt is vs else nc.vector
                            v3eng.tensor_sub(vt[:, 3, :, cs], d1, d3)
                        it += 1

                        for i in range(4):
                            for br, vt in ((0, vc), (1, vs)):
                                for dw in range(3):
                                    t_idx = ((j * 4 + i) * 2 + br) * 3 + dw
                                    nc.tensor.matmul(
                                        pss[i][:],
                                        wt[:, t_idx, :],
                                        vt[:, i, :, dw + 1 : dw + 65],
                                        start=(j == 0 and br == 0 and dw == 0),
                                        stop=(j == 4 and br == 1 and dw == 2),
                                    )

                        # previous chunk's inverse, deferred so it doesn't
                        # head-of-line-block this chunk's DVE/ACT gen work
                        if j == 2 and pending is not None:
                            emit_inverse(pending)
                            pending = None

                    pending = (b, r0, pss)
            if pending is not None:
                emit_inverse(pending)
    nc.finalize()
    return nc


def _get_module(reps=1, ht=HT):
    key = ("nc", reps, ht)
    if key not in _CACHE:
        _CACHE[key] = _build_module(reps, ht)
    return _CACHE[key]


def _host_weights(fc, mmdt=None):
    # fc: (2, O, C, kH, kW, G) -> winograd-H transform along kH, then pack
    # lhsT[p=(gp*64+c), t=((i*5+j)*2+br)*3+dw, o]
    import ml_dtypes

    Gw = np.array(
        [[1, 0, 0], [0.5, 0.5, 0.5], [0.5, -0.5, 0.5], [0, 0, 1]], np.float32
    )
    wt6 = np.einsum("iK,aocKVg->aociVg", Gw, fc)  # (2, O, C, 4, 3, G)
    wt7 = wt6.reshape(2, O, C, 4, 3, 5, 2)  # g -> (j, gp)
    # -> (gp, c, j, i, br, dw, o): j-major so the first chunk's weight
    # tiles land early in the startup DMA
    wt8 = np.transpose(wt7, (6, 2, 5, 3, 0, 4, 1))
    return np.ascontiguousarray(
        wt8.reshape(128, NT, 128).astype(ml_dtypes.bfloat16)
    )


def _host_kvec():
    kvec = np.zeros((128, 5), np.float32)
    for j in range(5):
        kvec[0:64, j] = (2 * j + 1) / TWO_PI
        kvec[64:128, j] = (2 * j + 2) / TWO_PI
    return kvec


def kernel(x, fouriercoeffs, bias):
    x = np.ascontiguousarray(np.asarray(x, dtype=np.float32))
    fc = np.asarray(fouriercoeffs, dtype=np.float32)
    w_host = _host_weights(fc)
    kvec = _host_kvec()
    biasv = np.ascontiguousarray(
        np.asarray(bias, dtype=np.float32).reshape(128, 1)
    )

    nc = _get_module()
    in_maps = [
        {"x": x[i * BS : (i + 1) * BS], "w": w_host, "kvec": kvec, "biasv": biasv}
        for i in range(N_CORES)
    ]
    res = run_bass_kernel_spmd(nc, in_maps, list(range(N_CORES))).results
    return np.concatenate([res[i]["y"] for i in range(N_CORES)], axis=0)
